# revision 44
# baseline (speedup 1.0000x reference)
"""Original baseline kernel (v1) — for throttle calibration."""
import sys, math

sys.path.insert(0, "/opt/trn_rl_repo")

import numpy as np
import ml_dtypes

import concourse.bacc as bacc
import concourse.bass as bass
import concourse.mybir as mybir
import concourse.tile as tile
from concourse.bass_utils import run_bass_kernel_spmd

BF16 = mybir.dt.bfloat16
F32 = mybir.dt.float32
NPBF16 = ml_dtypes.bfloat16

D_MODEL = 1024
D_HEAD = 64
HALF = D_HEAD // 2
ROPE_THETA = 10000.0
N_CORES = 8
C = 256  # channels per core (4 heads x 64)
SWAP32 = [i ^ 1 for i in range(32)]


def _body(nc, tc, L, pp, rtp, ptp, rip, osp):
    n_lt = L // 128
    n_qk = max(1, L // 512)
    qkw = min(512, L)
    qw = min(512, L)
    n_qch = L // qw

    xt_d = nc.dram_tensor("xt", [D_MODEL, L], BF16, kind="ExternalInput").ap()
    wq_d = nc.dram_tensor("wqt", [D_MODEL, C], BF16, kind="ExternalInput").ap()
    wk_d = nc.dram_tensor("wkt", [D_MODEL, C], BF16, kind="ExternalInput").ap()
    wv_d = nc.dram_tensor("wvt", [D_MODEL, C], BF16, kind="ExternalInput").ap()
    wo_d = nc.dram_tensor("wot", [C, D_MODEL], BF16, kind="ExternalInput").ap()
    cos_d = nc.dram_tensor("cosb", [128, L], BF16, kind="ExternalInput").ap()
    sin_d = nc.dram_tensor("ssin", [128, L], BF16, kind="ExternalInput").ap()
    mk_d = nc.dram_tensor("masks", [128, 128], BF16,
                          kind="ExternalInput").ap()
    out_d = nc.dram_tensor("out", [L, D_MODEL], BF16,
                           kind="ExternalOutput").ap()

    wq = pp.tile([128, 8, C], BF16)
    wk = pp.tile([128, 8, C], BF16)
    wv = pp.tile([128, 8, C], BF16)
    wo = pp.tile([128, 2, D_MODEL], BF16)
    cs = pp.tile([128, L], BF16)
    sn = pp.tile([128, L], BF16)
    mks = pp.tile([128, 128], BF16)
    ones = pp.tile([128, 64], BF16)
    n_ch = max(1, L // 512)
    chw = min(512, L)
    qt_c = [pp.tile([128, 2, chw], BF16, name=f"qt{i}") for i in range(n_ch)]
    kt_c = [pp.tile([128, 2, chw], BF16, name=f"ktc{i}") for i in range(n_ch)]
    vt_c = [pp.tile([128, chw // 128, C + 4], BF16, name=f"vt{i}")
            for i in range(n_ch)]
    at = pp.tile([128, 2, L], BF16)
    xts = [pp.tile([128, L], BF16, name=f"xt{i}") for i in range(8)]

    # three DMA queues in parallel, each ordered by consumption time: the
    # weights for the first matmuls lead two queues, x d-tiles round-robin
    # so xt[d] lands well before the dt-outer first chunk consumes it
    nc.sync.dma_start(out=wq[:], in_=wq_d.rearrange("(a p) c -> p a c", p=128))
    nc.scalar.dma_start(out=wk[:],
                        in_=wk_d.rearrange("(a p) c -> p a c", p=128))
    nc.gpsimd.dma_start(out=xts[0][:], in_=xt_d[0:128, :])
    engs = (nc.sync, nc.scalar, nc.gpsimd)
    for i in range(1, 8):
        engs[i % 3].dma_start(out=xts[i][:],
                              in_=xt_d[i * 128:(i + 1) * 128, :])
    nc.sync.dma_start(out=cs[:], in_=cos_d)
    nc.scalar.dma_start(out=sn[:], in_=sin_d)
    nc.gpsimd.dma_start(out=wv[:],
                        in_=wv_d.rearrange("(a p) c -> p a c", p=128))
    nc.sync.dma_start(out=wo[:], in_=wo_d.rearrange("(a p) e -> p a e", p=128))
    nc.scalar.dma_start(out=mks[:], in_=mk_d)
    nc.gpsimd.memset(ones[:], 1.0)
    for i in range(len(vt_c)):
        ov = vt_c[i][:, :, :].rearrange("p l (h x) -> p l h x", x=65)
        nc.gpsimd.memset(ov[:, :, :, 64], 1.0)

    with tc.tile_pool(name="qk_ps", bufs=6, space="PSUM") as qkps, \
         tc.tile_pool(name="v_ps", bufs=2, space="PSUM") as vps:
        for qc in range(n_qk):
            ls = qc * qkw
            ps = {}
            if qc == 0:
                # first chunk dt-OUTER: its 4 accumulation groups interleave
                # so the first matmul needs only xts[0] -- the PE starts as
                # soon as the first x d-tile lands instead of after all 8
                for nm, w in (("q", wq), ("k", wk)):
                    for ct in (0, 1):
                        ps[(nm, ct)] = qkps.tile(
                            [128, qkw], F32, tag="qkps",
                            name=f"ps_{nm}{ct}_{qc}")
                for dt_ in range(8):
                    for nm, w in (("q", wq), ("k", wk)):
                        for ct in (0, 1):
                            nc.tensor.matmul(
                                ps[(nm, ct)][:],
                                lhsT=w[:, dt_, ct * 128:ct * 128 + 128],
                                rhs=xts[dt_][:, ls:ls + qkw],
                                start=(dt_ == 0), stop=(dt_ == 7),
                                skip_group_check=True)
            else:
                for nm, w in (("q", wq), ("k", wk)):
                    for ct in (0, 1):
                        p = qkps.tile([128, qkw], F32, tag="qkps",
                                      name=f"ps_{nm}{ct}_{qc}")
                        for dt_ in range(8):
                            nc.tensor.matmul(
                                p[:],
                                lhsT=w[:, dt_, ct * 128:ct * 128 + 128],
                                rhs=xts[dt_][:, ls:ls + qkw],
                                start=(dt_ == 0), stop=(dt_ == 7))
                        ps[(nm, ct)] = p
            for nm, dstc in (("q", qt_c), ("k", kt_c)):
                dst = dstc[qc]
                for ct in (0, 1):
                    p = ps[(nm, ct)]
                    sh = rtp.tile([128, qkw], F32, tag="t",
                                  name=f"sh_{nm}{ct}{qc}")
                    t1 = rtp.tile([128, qkw], F32, tag="t",
                                  name=f"t1_{nm}{ct}{qc}")
                    t2 = rtp.tile([128, qkw], F32, tag="t",
                                  name=f"t2_{nm}{ct}{qc}")
                    nc.vector.stream_shuffle(sh[:], p[:], SWAP32)
                    nc.vector.tensor_mul(t1[:], p[:], cs[:, ls:ls + qkw])
                    nc.gpsimd.tensor_mul(t2[:], sh[:], sn[:, ls:ls + qkw])
                    nc.gpsimd.tensor_add(dst[:, ct, :], t1[:], t2[:])
            for lt in range(ls // 128, (ls + qkw) // 128):
                pv = vps.tile([128, C], F32, tag="vps", name=f"pv_{lt}")
                for dt_ in range(8):
                    nc.tensor.matmul(
                        pv[:],
                        lhsT=xts[dt_][:, lt * 128:lt * 128 + 128],
                        rhs=wv[:, dt_, :],
                        start=(dt_ == 0), stop=(dt_ == 7))
                ov = vt_c[lt // 4][:, lt % 4, :].rearrange(
                    "p (h x) -> p h x", x=65)[:, :, 0:64]
                nc.scalar.copy(ov, pv[:].rearrange("p (h x) -> p h x", x=64))

    scale = 1.0 / math.sqrt(D_HEAD)
    with tc.tile_pool(name="att_ps", bufs=2, space="PSUM") as atps, \
         tc.tile_pool(name="o_ps", bufs=2, space="PSUM") as ops_, \
         tc.tile_pool(name="riscr_p", bufs=4, space="DRAM") as scrp:
        for pair in range(2):
            for qc in range(n_qch):
                qs = qc * qw
                ktmax = (qs + qw) // 128
                po = ops_.tile([128, 1024], F32, tag="o", name=f"po_{pair}_{qc}")
                for kt in range(ktmax):
                    off = kt * 128 - qs
                    qlo = max(0, off)
                    kc, ko = kt // 4, (kt % 4) * 128
                    pt_ps = atps.tile([128, 1024], F32, tag="tps",
                                      name=f"pt_{pair}_{qc}_{kt}")
                    for hloc in range(2):
                        nc.tensor.matmul(
                            pt_ps[:, 512 * hloc + qlo:512 * hloc + qw],
                            lhsT=kt_c[kc][64 * hloc:64 * hloc + 64, pair,
                                          ko:ko + 128],
                            rhs=qt_c[qc][64 * hloc:64 * hloc + 64, pair,
                                         qlo:qw],
                            start=True, stop=True,
                            tile_position=(64 * hloc, 0),
                            skip_group_check=True)
                    pt_sb = ptp.tile([128, 1024], BF16, tag="p",
                                     name=f"ptsb_{pair}_{qc}_{kt}")
                    pv_ps = pt_ps[:, :].rearrange("p (h x) -> p h x", h=2)
                    pv_sb = pt_sb[:, :].rearrange("p (h x) -> p h x", h=2)
                    nc.scalar.activation(pv_sb[:, :, qlo:qw],
                                         pv_ps[:, :, qlo:qw],
                                         mybir.ActivationFunctionType.Exp,
                                         scale=scale)
                    if off >= 0:
                        for hloc in range(2):
                            nc.vector.tensor_mul(
                                pt_sb[:, 512 * hloc + qlo:512 * hloc + qlo + 128],
                                pt_sb[:, 512 * hloc + qlo:512 * hloc + qlo + 128],
                                mks[:, 0:128])
                    for hloc in range(2):
                        h = 2 * pair + hloc
                        nc.tensor.matmul(
                            po[0:65, 512 * hloc + qlo:512 * hloc + qw],
                            lhsT=vt_c[kc][:, kt % 4, 65 * h:65 * h + 65],
                            rhs=pt_sb[:, 512 * hloc + qlo:512 * hloc + qw],
                            start=(kt == 0), stop=(kt == ktmax - 1),
                            skip_group_check=True)
                rrow = rip.tile([1, 1024], F32, tag="ri",
                                name=f"rr_{pair}_{qc}")
                if qw == 512:
                    nc.vector.tensor_copy(rrow[:], po[64:65, :])
                else:
                    for hloc in range(2):
                        nc.vector.tensor_copy(
                            rrow[:, qw * hloc:qw * hloc + qw],
                            po[64:65, 512 * hloc:512 * hloc + qw])
                scrt = scrp.tile([1, 1024], F32, tag="scr",
                                 name=f"scr_{pair}_{qc}")
                scr = scrt[:, 0:2 * qw]
                nc.sync.dma_start(out=scr, in_=rrow[:, 0:2 * qw])
                pb = rip.tile([64, 1024], F32, tag="pb",
                              name=f"pb_{pair}_{qc}")
                nc.sync.dma_start(out=pb[:, 0:2 * qw],
                                  in_=scr.partition_broadcast(64))
                pbi = rip.tile([64, 1024], F32, tag="pbi",
                               name=f"pbi_{pair}_{qc}")
                nc.vector.reciprocal_approx_fast(out=pbi[:, 0:2 * qw],
                                                 in_=pb[:, 0:2 * qw])
                tm = rip.tile([64, 1024], BF16, tag="tm",
                              name=f"tm_{pair}_{qc}")
                if qw == 512:
                    nc.vector.tensor_mul(tm[:], po[0:64, :], pbi[:])
                else:
                    for hloc in range(2):
                        nc.vector.tensor_mul(
                            tm[:, 512 * hloc:512 * hloc + qw],
                            po[0:64, 512 * hloc:512 * hloc + qw],
                            pbi[:, qw * hloc:qw * hloc + qw])
                nc.vector.tensor_copy(at[0:64, pair, qs:qs + qw],
                                      tm[:, 0:qw])
                nc.sync.dma_start(out=at[64:128, pair, qs:qs + qw],
                                  in_=tm[:, 512:512 + qw])
    with tc.tile_pool(name="out_ps", bufs=2, space="PSUM") as outps:
        for qtl in range(n_lt):
            pout = outps.tile([128, 1024], F32, tag="outps",
                              name=f"pout_{qtl}")
            for ct in range(2):
                for eh in range(2):
                    nc.tensor.matmul(
                        pout[:, eh * 512:eh * 512 + 512],
                        lhsT=at[:, ct, qtl * 128:qtl * 128 + 128],
                        rhs=wo[:, ct, eh * 512:eh * 512 + 512],
                        start=(ct == 0), stop=(ct == 1),
                        skip_group_check=True)
            # bf16 staging (partials are summed in f32 on the host) and
            # round-robin DMA queues: the 4 MiB output drains in parallel
            # instead of serializing the final tiles on one queue
            stg = osp.tile([128, 1024], BF16, tag="stg", name=f"stg_{qtl}")
            nc.vector.tensor_copy(stg[:, 0:512], pout[:, 0:512])
            nc.scalar.copy(stg[:, 512:1024], pout[:, 512:1024])
            eng = (nc.sync, nc.scalar, nc.gpsimd)[qtl % 3]
            eng.dma_start(out=out_d[qtl * 128:qtl * 128 + 128, :],
                          in_=stg[:])


def build_nc(L=2048):
    assert L % 256 == 0
    nc = bacc.Bacc("TRN2", target_bir_lowering=False, debug=False,
                   num_devices=N_CORES)
    with tile.TileContext(nc) as tc:
        with tc.tile_pool(name="persist", bufs=1) as pp, \
             tc.tile_pool(name="ropet", bufs=6) as rtp, \
             tc.tile_pool(name="ptp", bufs=4) as ptp, \
             tc.tile_pool(name="rinvp", bufs=2) as rip, \
             tc.tile_pool(name="ostg", bufs=4) as osp:
            _body(nc, tc, L, pp, rtp, ptp, rip, osp)
    nc.compile()
    return nc


_NC_CACHE = {}


def _get_nc(L):
    if L not in _NC_CACHE:
        _NC_CACHE[L] = build_nc(L)
    return _NC_CACHE[L]


def make_inputs(x, token_positions, Wq, Wk, Wv, Wo):
    B, L, _ = x.shape
    pos = np.asarray(token_positions).astype(np.float64)
    S = ROPE_THETA ** (-2.0 / D_HEAD)
    thetas = S ** np.arange(HALF, dtype=np.float64)
    ang = pos[:, None] * thetas[None, :]
    cosL = np.cos(ang).T
    sinL = np.sin(ang).T
    cosb = np.empty((128, L), dtype=np.float64)
    ssin = np.empty((128, L), dtype=np.float64)
    for p in range(128):
        i = (p % 64) // 2
        cosb[p] = cosL[i]
        ssin[p] = -sinL[i] if (p % 2 == 0) else sinL[i]
    cosb = cosb.astype(NPBF16)
    ssin = ssin.astype(NPBF16)

    r = np.arange(128)[:, None]
    col = np.arange(128)[None, :]
    masks = (col >= r).astype(NPBF16)

    xts = [np.ascontiguousarray(x[b].astype(NPBF16).T) for b in range(B)]
    in_maps = []
    shard_cache = {}
    for core in range(N_CORES):
        b, hg = core // 4, core % 4
        if hg not in shard_cache:
            rows = slice(hg * 256, hg * 256 + 256)
            shard_cache[hg] = {
                "wqt": np.ascontiguousarray(Wq[rows].astype(NPBF16).T),
                "wkt": np.ascontiguousarray(Wk[rows].astype(NPBF16).T),
                "wvt": np.ascontiguousarray(Wv[rows].astype(NPBF16).T),
                "wot": np.ascontiguousarray(Wo[:, rows].astype(NPBF16).T),
            }
        m = dict(shard_cache[hg])
        m["xt"] = xts[b]
        m["cosb"] = cosb
        m["ssin"] = ssin
        m["masks"] = masks
        in_maps.append(m)
    return in_maps


def kernel(x, token_positions, Wq, Wk, Wv, Wo):
    x = np.asarray(x); Wq = np.asarray(Wq); Wk = np.asarray(Wk)
    Wv = np.asarray(Wv); Wo = np.asarray(Wo)
    B, L, _ = x.shape
    nc = _get_nc(L)
    in_maps = make_inputs(x, token_positions, Wq, Wk, Wv, Wo)
    res = run_bass_kernel_spmd(nc, in_maps, core_ids=list(range(N_CORES)))
    out = np.zeros((B, L, D_MODEL), dtype=np.float32)
    for core in range(N_CORES):
        out[core // 4] += res.results[core]["out"]
    return out


# revision 46
# speedup vs baseline: 1.0248x; 1.0248x over previous
"""Original baseline kernel (v1) — for throttle calibration."""
import sys, math

sys.path.insert(0, "/opt/trn_rl_repo")

import numpy as np
import ml_dtypes

import concourse.bacc as bacc
import concourse.bass as bass
import concourse.mybir as mybir
import concourse.tile as tile
from concourse.bass_utils import run_bass_kernel_spmd

BF16 = mybir.dt.bfloat16
F32 = mybir.dt.float32
NPBF16 = ml_dtypes.bfloat16

D_MODEL = 1024
D_HEAD = 64
HALF = D_HEAD // 2
ROPE_THETA = 10000.0
N_CORES = 8
C = 256  # channels per core (4 heads x 64)
SWAP32 = [i ^ 1 for i in range(32)]


def _body(nc, tc, L, pp, rtp, ptp, rip, osp):
    n_lt = L // 128
    n_qk = max(1, L // 512)
    qkw = min(512, L)
    qw = min(512, L)
    n_qch = L // qw

    xt_d = nc.dram_tensor("xt", [D_MODEL, L], BF16, kind="ExternalInput").ap()
    wq_d = nc.dram_tensor("wqt", [D_MODEL, C], BF16, kind="ExternalInput").ap()
    wk_d = nc.dram_tensor("wkt", [D_MODEL, C], BF16, kind="ExternalInput").ap()
    wv_d = nc.dram_tensor("wvt", [D_MODEL, C], BF16, kind="ExternalInput").ap()
    wo_d = nc.dram_tensor("wot", [C, D_MODEL], BF16, kind="ExternalInput").ap()
    cos_d = nc.dram_tensor("cosb", [128, L], BF16, kind="ExternalInput").ap()
    sin_d = nc.dram_tensor("ssin", [128, L], BF16, kind="ExternalInput").ap()
    mk_d = nc.dram_tensor("masks", [128, 128], BF16,
                          kind="ExternalInput").ap()
    out_d = nc.dram_tensor("out", [L, D_MODEL], BF16,
                           kind="ExternalOutput").ap()

    wq = pp.tile([128, 8, C], BF16)
    wk = pp.tile([128, 8, C], BF16)
    wv = pp.tile([128, 8, C], BF16)
    wo = pp.tile([128, 2, D_MODEL], BF16)
    cs = pp.tile([128, L], BF16)
    sn = pp.tile([128, L], BF16)
    mks = pp.tile([128, 128], BF16)
    ones = pp.tile([128, 64], BF16)
    n_ch = max(1, L // 512)
    chw = min(512, L)
    qt_c = [pp.tile([128, 2, chw], BF16, name=f"qt{i}") for i in range(n_ch)]
    kt_c = [pp.tile([128, 2, chw], BF16, name=f"ktc{i}") for i in range(n_ch)]
    vt_c = [pp.tile([128, chw // 128, C + 4], BF16, name=f"vt{i}")
            for i in range(n_ch)]
    at = pp.tile([128, 2, L], BF16)
    xts = [pp.tile([128, L], BF16, name=f"xt{i}") for i in range(8)]

    # ONE serial DMA queue in consumption order: queue serialization
    # dedicates HBM bandwidth to the next-needed tensor (a parallel spread
    # delays the critical first tiles -- measured, twice)
    nc.sync.dma_start(out=wq[:], in_=wq_d.rearrange("(a p) c -> p a c", p=128))
    nc.sync.dma_start(out=xts[0][:], in_=xt_d[0:128, :])
    nc.sync.dma_start(out=xts[1][:], in_=xt_d[128:256, :])
    nc.sync.dma_start(out=wk[:], in_=wk_d.rearrange("(a p) c -> p a c", p=128))
    for i in range(2, 8):
        nc.sync.dma_start(out=xts[i][:], in_=xt_d[i * 128:(i + 1) * 128, :])
    nc.sync.dma_start(out=cs[:], in_=cos_d)
    nc.sync.dma_start(out=sn[:], in_=sin_d)
    nc.sync.dma_start(out=wv[:], in_=wv_d.rearrange("(a p) c -> p a c", p=128))
    nc.sync.dma_start(out=wo[:], in_=wo_d.rearrange("(a p) e -> p a e", p=128))
    nc.sync.dma_start(out=mks[:], in_=mk_d)
    nc.gpsimd.memset(ones[:], 1.0)
    for i in range(len(vt_c)):
        ov = vt_c[i][:, :, :].rearrange("p l (h x) -> p l h x", x=65)
        nc.gpsimd.memset(ov[:, :, :, 64], 1.0)

    with tc.tile_pool(name="qk_ps", bufs=6, space="PSUM") as qkps, \
         tc.tile_pool(name="v_ps", bufs=2, space="PSUM") as vps:
        for qc in range(n_qk):
            ls = qc * qkw
            ps = {}
            if qc == 0:
                # first chunk dt-OUTER: its 4 accumulation groups interleave
                # so the first matmul needs only xts[0] -- the PE starts as
                # soon as the first x d-tile lands instead of after all 8
                for nm, w in (("q", wq), ("k", wk)):
                    for ct in (0, 1):
                        ps[(nm, ct)] = qkps.tile(
                            [128, qkw], F32, tag="qkps",
                            name=f"ps_{nm}{ct}_{qc}")
                for dt_ in range(8):
                    for nm, w in (("q", wq), ("k", wk)):
                        for ct in (0, 1):
                            nc.tensor.matmul(
                                ps[(nm, ct)][:],
                                lhsT=w[:, dt_, ct * 128:ct * 128 + 128],
                                rhs=xts[dt_][:, ls:ls + qkw],
                                start=(dt_ == 0), stop=(dt_ == 7),
                                skip_group_check=True)
            else:
                for nm, w in (("q", wq), ("k", wk)):
                    for ct in (0, 1):
                        p = qkps.tile([128, qkw], F32, tag="qkps",
                                      name=f"ps_{nm}{ct}_{qc}")
                        for dt_ in range(8):
                            nc.tensor.matmul(
                                p[:],
                                lhsT=w[:, dt_, ct * 128:ct * 128 + 128],
                                rhs=xts[dt_][:, ls:ls + qkw],
                                start=(dt_ == 0), stop=(dt_ == 7))
                        ps[(nm, ct)] = p
            for nm, dstc in (("q", qt_c), ("k", kt_c)):
                dst = dstc[qc]
                for ct in (0, 1):
                    p = ps[(nm, ct)]
                    sh = rtp.tile([128, qkw], F32, tag="t",
                                  name=f"sh_{nm}{ct}{qc}")
                    t1 = rtp.tile([128, qkw], F32, tag="t",
                                  name=f"t1_{nm}{ct}{qc}")
                    t2 = rtp.tile([128, qkw], F32, tag="t",
                                  name=f"t2_{nm}{ct}{qc}")
                    nc.vector.stream_shuffle(sh[:], p[:], SWAP32)
                    nc.vector.tensor_mul(t1[:], p[:], cs[:, ls:ls + qkw])
                    nc.gpsimd.tensor_mul(t2[:], sh[:], sn[:, ls:ls + qkw])
                    nc.gpsimd.tensor_add(dst[:, ct, :], t1[:], t2[:])
            for lt in range(ls // 128, (ls + qkw) // 128):
                pv = vps.tile([128, C], F32, tag="vps", name=f"pv_{lt}")
                for dt_ in range(8):
                    nc.tensor.matmul(
                        pv[:],
                        lhsT=xts[dt_][:, lt * 128:lt * 128 + 128],
                        rhs=wv[:, dt_, :],
                        start=(dt_ == 0), stop=(dt_ == 7))
                ov = vt_c[lt // 4][:, lt % 4, :].rearrange(
                    "p (h x) -> p h x", x=65)[:, :, 0:64]
                nc.scalar.copy(ov, pv[:].rearrange("p (h x) -> p h x", x=64))

    scale = 1.0 / math.sqrt(D_HEAD)
    with tc.tile_pool(name="att_ps", bufs=2, space="PSUM") as atps, \
         tc.tile_pool(name="o_ps", bufs=2, space="PSUM") as ops_, \
         tc.tile_pool(name="riscr_p", bufs=4, space="DRAM") as scrp:
        for pair in range(2):
            for qc in range(n_qch):
                qs = qc * qw
                ktmax = (qs + qw) // 128
                po = ops_.tile([128, 1024], F32, tag="o", name=f"po_{pair}_{qc}")
                for kt in range(ktmax):
                    off = kt * 128 - qs
                    qlo = max(0, off)
                    kc, ko = kt // 4, (kt % 4) * 128
                    pt_ps = atps.tile([128, 1024], F32, tag="tps",
                                      name=f"pt_{pair}_{qc}_{kt}")
                    for hloc in range(2):
                        nc.tensor.matmul(
                            pt_ps[:, 512 * hloc + qlo:512 * hloc + qw],
                            lhsT=kt_c[kc][64 * hloc:64 * hloc + 64, pair,
                                          ko:ko + 128],
                            rhs=qt_c[qc][64 * hloc:64 * hloc + 64, pair,
                                         qlo:qw],
                            start=True, stop=True,
                            tile_position=(64 * hloc, 0),
                            skip_group_check=True)
                    pt_sb = ptp.tile([128, 1024], BF16, tag="p",
                                     name=f"ptsb_{pair}_{qc}_{kt}")
                    pv_ps = pt_ps[:, :].rearrange("p (h x) -> p h x", h=2)
                    pv_sb = pt_sb[:, :].rearrange("p (h x) -> p h x", h=2)
                    nc.scalar.activation(pv_sb[:, :, qlo:qw],
                                         pv_ps[:, :, qlo:qw],
                                         mybir.ActivationFunctionType.Exp,
                                         scale=scale)
                    if off >= 0:
                        for hloc in range(2):
                            nc.vector.tensor_mul(
                                pt_sb[:, 512 * hloc + qlo:512 * hloc + qlo + 128],
                                pt_sb[:, 512 * hloc + qlo:512 * hloc + qlo + 128],
                                mks[:, 0:128])
                    for hloc in range(2):
                        h = 2 * pair + hloc
                        nc.tensor.matmul(
                            po[0:65, 512 * hloc + qlo:512 * hloc + qw],
                            lhsT=vt_c[kc][:, kt % 4, 65 * h:65 * h + 65],
                            rhs=pt_sb[:, 512 * hloc + qlo:512 * hloc + qw],
                            start=(kt == 0), stop=(kt == ktmax - 1),
                            skip_group_check=True)
                rrow = rip.tile([1, 1024], F32, tag="ri",
                                name=f"rr_{pair}_{qc}")
                if qw == 512:
                    nc.vector.tensor_copy(rrow[:], po[64:65, :])
                else:
                    for hloc in range(2):
                        nc.vector.tensor_copy(
                            rrow[:, qw * hloc:qw * hloc + qw],
                            po[64:65, 512 * hloc:512 * hloc + qw])
                scrt = scrp.tile([1, 1024], F32, tag="scr",
                                 name=f"scr_{pair}_{qc}")
                scr = scrt[:, 0:2 * qw]
                nc.sync.dma_start(out=scr, in_=rrow[:, 0:2 * qw])
                pb = rip.tile([64, 1024], F32, tag="pb",
                              name=f"pb_{pair}_{qc}")
                nc.sync.dma_start(out=pb[:, 0:2 * qw],
                                  in_=scr.partition_broadcast(64))
                pbi = rip.tile([64, 1024], F32, tag="pbi",
                               name=f"pbi_{pair}_{qc}")
                nc.vector.reciprocal_approx_fast(out=pbi[:, 0:2 * qw],
                                                 in_=pb[:, 0:2 * qw])
                tm = rip.tile([64, 1024], BF16, tag="tm",
                              name=f"tm_{pair}_{qc}")
                if qw == 512:
                    nc.vector.tensor_mul(tm[:], po[0:64, :], pbi[:])
                else:
                    for hloc in range(2):
                        nc.vector.tensor_mul(
                            tm[:, 512 * hloc:512 * hloc + qw],
                            po[0:64, 512 * hloc:512 * hloc + qw],
                            pbi[:, qw * hloc:qw * hloc + qw])
                nc.vector.tensor_copy(at[0:64, pair, qs:qs + qw],
                                      tm[:, 0:qw])
                nc.sync.dma_start(out=at[64:128, pair, qs:qs + qw],
                                  in_=tm[:, 512:512 + qw])
    with tc.tile_pool(name="out_ps", bufs=2, space="PSUM") as outps:
        for qtl in range(n_lt):
            pout = outps.tile([128, 1024], F32, tag="outps",
                              name=f"pout_{qtl}")
            for ct in range(2):
                for eh in range(2):
                    nc.tensor.matmul(
                        pout[:, eh * 512:eh * 512 + 512],
                        lhsT=at[:, ct, qtl * 128:qtl * 128 + 128],
                        rhs=wo[:, ct, eh * 512:eh * 512 + 512],
                        start=(ct == 0), stop=(ct == 1),
                        skip_group_check=True)
            # bf16 staging (partials are summed in f32 on the host) and
            # round-robin DMA queues: the 4 MiB output drains in parallel
            # instead of serializing the final tiles on one queue
            stg = osp.tile([128, 1024], BF16, tag="stg", name=f"stg_{qtl}")
            nc.vector.tensor_copy(stg[:, 0:512], pout[:, 0:512])
            nc.scalar.copy(stg[:, 512:1024], pout[:, 512:1024])
            eng = (nc.sync, nc.scalar, nc.gpsimd)[qtl % 3]
            eng.dma_start(out=out_d[qtl * 128:qtl * 128 + 128, :],
                          in_=stg[:])


def build_nc(L=2048):
    assert L % 256 == 0
    nc = bacc.Bacc("TRN2", target_bir_lowering=False, debug=False,
                   num_devices=N_CORES)
    with tile.TileContext(nc) as tc:
        with tc.tile_pool(name="persist", bufs=1) as pp, \
             tc.tile_pool(name="ropet", bufs=6) as rtp, \
             tc.tile_pool(name="ptp", bufs=3) as ptp, \
             tc.tile_pool(name="rinvp", bufs=2) as rip, \
             tc.tile_pool(name="ostg", bufs=3) as osp:
            _body(nc, tc, L, pp, rtp, ptp, rip, osp)
    nc.compile()
    return nc


_NC_CACHE = {}


def _get_nc(L):
    if L not in _NC_CACHE:
        _NC_CACHE[L] = build_nc(L)
    return _NC_CACHE[L]


def make_inputs(x, token_positions, Wq, Wk, Wv, Wo):
    B, L, _ = x.shape
    pos = np.asarray(token_positions).astype(np.float64)
    S = ROPE_THETA ** (-2.0 / D_HEAD)
    thetas = S ** np.arange(HALF, dtype=np.float64)
    ang = pos[:, None] * thetas[None, :]
    cosL = np.cos(ang).T
    sinL = np.sin(ang).T
    cosb = np.empty((128, L), dtype=np.float64)
    ssin = np.empty((128, L), dtype=np.float64)
    for p in range(128):
        i = (p % 64) // 2
        cosb[p] = cosL[i]
        ssin[p] = -sinL[i] if (p % 2 == 0) else sinL[i]
    cosb = cosb.astype(NPBF16)
    ssin = ssin.astype(NPBF16)

    r = np.arange(128)[:, None]
    col = np.arange(128)[None, :]
    masks = (col >= r).astype(NPBF16)

    xts = [np.ascontiguousarray(x[b].astype(NPBF16).T) for b in range(B)]
    in_maps = []
    shard_cache = {}
    for core in range(N_CORES):
        b, hg = core // 4, core % 4
        if hg not in shard_cache:
            rows = slice(hg * 256, hg * 256 + 256)
            shard_cache[hg] = {
                "wqt": np.ascontiguousarray(Wq[rows].astype(NPBF16).T),
                "wkt": np.ascontiguousarray(Wk[rows].astype(NPBF16).T),
                "wvt": np.ascontiguousarray(Wv[rows].astype(NPBF16).T),
                "wot": np.ascontiguousarray(Wo[:, rows].astype(NPBF16).T),
            }
        m = dict(shard_cache[hg])
        m["xt"] = xts[b]
        m["cosb"] = cosb
        m["ssin"] = ssin
        m["masks"] = masks
        in_maps.append(m)
    return in_maps


def kernel(x, token_positions, Wq, Wk, Wv, Wo):
    x = np.asarray(x); Wq = np.asarray(Wq); Wk = np.asarray(Wk)
    Wv = np.asarray(Wv); Wo = np.asarray(Wo)
    B, L, _ = x.shape
    nc = _get_nc(L)
    in_maps = make_inputs(x, token_positions, Wq, Wk, Wv, Wo)
    res = run_bass_kernel_spmd(nc, in_maps, core_ids=list(range(N_CORES)))
    out = np.zeros((B, L, D_MODEL), dtype=np.float32)
    for core in range(N_CORES):
        out[core // 4] += res.results[core]["out"]
    return out
